# revision 1
# baseline (speedup 1.0000x reference)
"""Trainium2 Bass kernel for nn_BaseModel_7885559955990 (gnn_message_passing).

Model: 2 tiny GCN layers on a 1024-node graph -> flatten to v[16384] ->
relu(v @ L1_w[16384,16384] + L1_b) -> sigmoid(. @ L2_w[16384,32] + L2_b).

Distribution (8 cores, tensor-parallel per the sharding hint):
  - L1_w is sharded column-wise: core c computes v1_c = relu(v @ L1_w[:, c*2048:(c+1)*2048] + b_c)
  - L2_w is sharded row-wise:    core c computes partial_c = v1_c @ L2_w[c*2048:(c+1)*2048, :]
  - unshard = sum partials over cores, + L2_b, sigmoid  (32 floats, done host-side)
  - GCN layers are tiny and replicated on every core.

The graph operator (degree-normalized adjacency with self loops) depends only
on the edge-list input; it is densified host-side into AT[s, d] (4 MB) so the
message-passing aggregation runs as dense matmuls on the tensor engine.

The dominant cost is streaming the 128 MiB/core L1 slice from HBM
(~360 GB/s/core). The vector-matrix product uses v-chunks as the [128,1]
stationary operand so the PE streams weight columns at line rate.

Precision modes for the L1 stream (MODE):
  fp32  - exact; PE-bound (fp32 streams at 4 cyc/row): ~440 us
  f32r  - fp32 data, single-pass reduced-precision matmul: ~DMA roofline
  bf16  - bf16 weights: half the HBM traffic, ~2x faster than roofline
  split - W and v split into bf16 hi+lo pairs (3 matmul passes); same HBM
          bytes as fp32 but full-rate streaming -> DMA roofline with ~1e-6 err
"""

import numpy as np
import ml_dtypes
from contextlib import ExitStack

import concourse.bacc as bacc
import concourse.tile as tile
from concourse import mybir
from concourse.bass_utils import run_bass_kernel_spmd

F32 = mybir.dt.float32
F32R = mybir.dt.float32r
BF16 = mybir.dt.bfloat16
AF = mybir.ActivationFunctionType

N_CORES = 8
N_NODES = 1024
C = 16                    # GCN channel width
M = N_NODES * C           # 16384 flattened width
COLS = M // N_CORES       # 2048 L1 columns per core
N_OUT = 32
NK = M // 128             # 128 contraction chunks of 128

MODE = "split"            # default; see module docstring
TRACE = False             # set True (module-level) to profile; result in LAST_RESULT
LAST_RESULT = None

_MODE_CFG = {
    #        stream_dt, agg_dt, cpd (128-row chunks per DMA), split
    "fp32":  (F32,  F32,  2, False),
    "f32r":  (F32R, F32,  2, False),
    "bf16":  (BF16, BF16, 4, False),
    "split": (BF16, F32,  2, True),
}

# All DRAM tensors streamed at rate are pre-tiled on the host into
# partition-major [128, ...] layout so every dma_start is a plain 2D AP —
# 3D/rearranged APs defeat the 16-engine descriptor spray (measured
# 17 GB/s vs 287 GB/s per core).

_program_cache = {}


def _build(mode, repeat=1):
    # repeat > 1 duplicates the weight-stream phase (timing builds only):
    # wall-slope between two repeat values isolates the steady-state
    # stream+matmul rate, cancelling RPC overhead and kernel prefix/tail.
    stream_dt, agg_dt, cpd, split = _MODE_CFG[mode]
    np_stream = ml_dtypes.bfloat16 if stream_dt == BF16 else np.float32
    np_agg = ml_dtypes.bfloat16 if agg_dt == BF16 else np.float32

    nc = bacc.Bacc("TRN2", target_bir_lowering=False, debug=False,
                   num_devices=N_CORES)

    # ---- DRAM tensors (per-core views; replicated unless noted).
    # at/l1w/l2w are host-pre-tiled partition-major (see _prep_inputs).
    at = nc.dram_tensor("at", [128, 8 * N_NODES], agg_dt, kind="ExternalInput").ap()
    xt = nc.dram_tensor("xt", [C, N_NODES], F32, kind="ExternalInput").ap()
    w1 = nc.dram_tensor("w1", [C, C], F32, kind="ExternalInput").ap()
    b1 = nc.dram_tensor("b1", [C, 1], F32, kind="ExternalInput").ap()
    w2 = nc.dram_tensor("w2", [C, C], F32, kind="ExternalInput").ap()
    b2 = nc.dram_tensor("b2", [C, 1], F32, kind="ExternalInput").ap()
    sub = 2 if split else 1       # sub-chunks (hi/lo) per 128-row chunk
    # +8 KB pad per partition row: a power-of-two row stride aliases DRAM
    # banks (measured 228 -> 384 GB/s/core on the 128 MB stream)
    pad = 8192 // (2 if stream_dt == BF16 else 4)
    l1w = nc.dram_tensor("l1w", [128, NK * sub * COLS + pad], stream_dt,
                         kind="ExternalInput").ap()
    l1bt = nc.dram_tensor("l1bt", [128, COLS // 128], F32, kind="ExternalInput").ap()
    l2w = nc.dram_tensor("l2w", [128, (COLS // 128) * N_OUT], F32,
                         kind="ExternalInput").ap()
    out = nc.dram_tensor("out", [1, N_OUT], F32, kind="ExternalOutput").ap()

    n_vj = COLS // 128            # 16 v1 chunks
    n_ng = COLS // 512            # 4 psum bank groups for the big matmul
    ndma = NK // cpd              # big-stream DMA count

    with tile.TileContext(nc) as tc, ExitStack() as ctx:
        const = ctx.enter_context(tc.tile_pool(name="const", bufs=1))
        small = ctx.enter_context(tc.tile_pool(name="small", bufs=1))
        wpool = ctx.enter_context(tc.tile_pool(name="wpool", bufs=8))
        dpool = ctx.enter_context(tc.tile_pool(name="dpool", bufs=1, space="DRAM"))

        # ---- constant loads (issued first so they beat the weight stream
        # into the DMA queues)
        at_sb = const.tile([128, 8 * N_NODES], agg_dt, tag="at")
        nc.sync.dma_start(at_sb[:, :], at)
        xt_sb = const.tile([C, N_NODES], F32, tag="xt")
        nc.sync.dma_start(xt_sb[:, :], xt)
        w1_sb = const.tile([C, C], F32, tag="w1")
        nc.sync.dma_start(w1_sb[:, :], w1)
        b1_sb = const.tile([C, 1], F32, tag="b1")
        nc.sync.dma_start(b1_sb[:, :], b1)
        w2_sb = const.tile([C, C], F32, tag="w2")
        nc.sync.dma_start(w2_sb[:, :], w2)
        b2_sb = const.tile([C, 1], F32, tag="b2")
        nc.sync.dma_start(b2_sb[:, :], b2)
        l1bt_sb = const.tile([128, n_vj], F32, tag="l1bt")
        nc.sync.dma_start(l1bt_sb[:, :], l1bt)
        l2w_sb = const.tile([128, n_vj * N_OUT], F32, tag="l2w")
        nc.sync.dma_start(l2w_sb[:, :], l2w)

        # ---- GCN: two layers of  hT' = relu( (AT.T-aggregated (h W)) + b )
        # h is kept transposed: [16 channels (partitions), 1024 nodes].
        def gcn_layer(h_in, w_sb, b_sb, psz, psh, zpool, hpool, li):
            # z = h @ W, built node-tile-major: z_i [128 nodes, 16]
            z_tiles = []
            for i in range(8):
                zps = psz.tile([128, C], F32, tag="zps")
                nc.tensor.matmul(zps[:, :], h_in[:, 128 * i:128 * (i + 1)],
                                 w_sb[:, :], start=True, stop=True)
                z_sb = zpool.tile([128, C], agg_dt, tag=f"z{li}_{i}")
                nc.vector.tensor_copy(z_sb[:, :], zps[:, :])
                z_tiles.append(z_sb)
            # aggregate: outT[c, d] = sum_s z[s, c] * AT[s, d]
            hps = psh.tile([C, N_NODES], F32, tag="hps")
            for i in range(8):
                for hh in range(2):
                    nc.tensor.matmul(
                        hps[:, 512 * hh:512 * (hh + 1)],
                        z_tiles[i][:, :],
                        at_sb[:, 1024 * i + 512 * hh:1024 * i + 512 * (hh + 1)],
                        start=(i == 0), stop=(i == 7),
                    )
            h_out = hpool.tile([C, N_NODES], F32, tag=f"h{li}")
            nc.scalar.activation(h_out[:, :], hps[:, :], AF.Relu, bias=b_sb[:, :])
            return h_out

        with tc.tile_pool(name="psz", bufs=2, space="PSUM") as psz, \
             tc.tile_pool(name="psh", bufs=2, space="PSUM") as psh, \
             tc.tile_pool(name="zpool", bufs=1) as zpool, \
             tc.tile_pool(name="hpool", bufs=1) as hpool:
            h1 = gcn_layer(xt_sb, w1_sb, b1_sb, psz, psh, zpool, hpool, 1)
            h2 = gcn_layer(h1, w2_sb, b2_sb, psz, psh, zpool, hpool, 2)

            # ---- vcol: v-chunks as stationary columns. vcol[16a+c, k] = v[128k+16a+c]
            # = h2[8k+a, c] = h2T[c, 8k+a]
            vcol = small.tile([128, NK], F32, tag="vcol")
            h2v = h2[:, :].rearrange("c (k a) -> c k a", a=8)
            for a in range(8):
                nc.gpsimd.dma_start(vcol[16 * a:16 * (a + 1), :], h2v[:, :, a])

        if split:
            vhi = small.tile([128, NK], BF16, tag="vhi")
            nc.vector.tensor_copy(vhi[:, :], vcol[:, :])
            vhi_f = small.tile([128, NK], F32, tag="vhif")
            nc.vector.tensor_copy(vhi_f[:, :], vhi[:, :])
            vlo_f = small.tile([128, NK], F32, tag="vlof")
            nc.vector.tensor_sub(vlo_f[:, :], vcol[:, :], vhi_f[:, :])
            vlo = small.tile([128, NK], BF16, tag="vlo")
            nc.vector.tensor_copy(vlo[:, :], vlo_f[:, :])
            # passes: (stationary vec, hi/lo weight sub-chunk)
            passes = [(vhi, 0), (vlo, 0), (vhi, 1)]
        elif stream_dt == F32:
            passes = [(vcol, 0)]
        else:
            vs = small.tile([128, NK], stream_dt, tag="vs")
            nc.vector.tensor_copy(vs[:, :], vcol[:, :])
            passes = [(vs, 0)]

        # ---- big matmul: vps[0, n] = sum_k v[k] * L1[k, n]
        with tc.tile_pool(name="psv", bufs=1, space="PSUM") as psv, \
             tc.tile_pool(name="ps32", bufs=1, space="PSUM") as ps32:
            vps = psv.tile([1, COLS], F32, tag="vps")
            wfree = COLS * sub * cpd     # tile free elems per DMA
            for rep in range(repeat):
                for t in range(ndma):
                    wt = wpool.tile([128, wfree], stream_dt, tag="w")
                    nc.sync.dma_start(wt[:, :], l1w[:, wfree * t:wfree * (t + 1)])
                    for cc in range(cpd):
                        k = cpd * t + cc
                        for j in range(n_ng):
                            for si, (vv, wi) in enumerate(passes):
                                base = (sub * cc + wi) * 2048
                                nc.tensor.matmul(
                                    vps[0:1, 512 * j:512 * (j + 1)],
                                    vv[:, k:k + 1],
                                    wt[:, base + 512 * j:base + 512 * (j + 1)],
                                    start=(k == 0 and si == 0 and rep == 0),
                                    stop=(k == NK - 1 and si == len(passes) - 1
                                          and rep == repeat - 1),
                                )

            # ---- tail: v1 = relu(vps + b), re-laid out to [128, 16] via DRAM bounce
            v1row = small.tile([1, COLS], F32, tag="v1row")
            nc.scalar.copy(v1row[:, :], vps[0:1, :])
            dscratch = dpool.tile([1, COLS], F32, tag="dscratch")
            nc.gpsimd.dma_start(dscratch[:, :], v1row[:, :])
            v1t = small.tile([128, n_vj], F32, tag="v1t")
            nc.gpsimd.dma_start(
                v1t[:, :], dscratch[:, :].rearrange("o (j p) -> p (o j)", p=128))
            nc.vector.tensor_add(v1t[:, :], v1t[:, :], l1bt_sb[:, :])
            nc.vector.tensor_relu(v1t[:, :], v1t[:, :])

            # ---- second matmul: partial[1, 32] = sum_j v1_j^T @ L2_j
            p32 = ps32.tile([1, N_OUT], F32, tag="p32")
            for j in range(n_vj):
                nc.tensor.matmul(
                    p32[0:1, :], v1t[:, j:j + 1],
                    l2w_sb[:, N_OUT * j:N_OUT * (j + 1)],
                    start=(j == 0), stop=(j == n_vj - 1),
                )
            out_sb = small.tile([1, N_OUT], F32, tag="out")
            nc.vector.tensor_copy(out_sb[:, :], p32[0:1, :])
            nc.sync.dma_start(out, out_sb[:, :])

    nc.compile()
    return nc


def _host_adjacency(edge):
    """Dense AT[s, d] = sum over (self-looped, deg-normalized) edges s->d."""
    src = edge[0].astype(np.int64)
    dst = edge[1].astype(np.int64)
    loop = np.arange(N_NODES, dtype=np.int64)
    s = np.concatenate([src, loop])
    d = np.concatenate([dst, loop])
    deg = np.bincount(d, minlength=N_NODES).astype(np.float32)
    dinv = np.where(deg > 0, deg, np.float32(1.0)) ** np.float32(-0.5)
    norm = (dinv[s] * dinv[d]).astype(np.float32)
    at = np.zeros((N_NODES, N_NODES), np.float32)
    np.add.at(at, (s, d), norm)
    return at


def _prep_inputs(x, edge, W1, b1, W2, b2, L1_w, L1_b, L2_w, mode):
    stream_dt, agg_dt, cpd, split = _MODE_CFG[mode]
    np_stream = ml_dtypes.bfloat16 if stream_dt == BF16 else np.float32
    np_agg = ml_dtypes.bfloat16 if agg_dt == BF16 else np.float32

    # partition-major tiling: AT [1024,1024] -> [128, 8*1024] with
    # at_t[p, 1024*i + d] = AT[128*i + p, d]
    at = _host_adjacency(edge).astype(np_agg)
    at = np.ascontiguousarray(
        at.reshape(8, 128, N_NODES).transpose(1, 0, 2).reshape(128, 8 * N_NODES))
    xt = np.ascontiguousarray(np.asarray(x, np.float32).T)
    w1 = np.ascontiguousarray(np.asarray(W1, np.float32))
    b1v = np.asarray(b1, np.float32).reshape(C, 1).copy()
    w2 = np.ascontiguousarray(np.asarray(W2, np.float32))
    b2v = np.asarray(b2, np.float32).reshape(C, 1).copy()
    L1_w = np.asarray(L1_w, np.float32)
    L1_b = np.asarray(L1_b, np.float32)
    L2_w = np.asarray(L2_w, np.float32)

    in_maps = []
    for c in range(N_CORES):
        sl = slice(COLS * c, COLS * (c + 1))
        wsl = np.ascontiguousarray(L1_w[:, sl])
        pad = 8192 // (2 if np_stream == ml_dtypes.bfloat16 else 4)
        if split:
            hi = wsl.astype(ml_dtypes.bfloat16)
            lo = (wsl - hi.astype(np.float32)).astype(ml_dtypes.bfloat16)
            # partition-major, k-major then hi/lo:
            # l1[p, (2k+s)*2048 + n] = (hi if s==0 else lo)[128k+p, n]
            body = np.empty((NK, 2, 128, COLS), ml_dtypes.bfloat16)
            body[:, 0] = hi.reshape(NK, 128, COLS)
            body[:, 1] = lo.reshape(NK, 128, COLS)
            body = body.transpose(2, 0, 1, 3).reshape(128, NK * 2 * COLS)
        else:
            # l1[p, 2048k + n] = Wslice[128k + p, n]
            body = (wsl.astype(np_stream).reshape(NK, 128, COLS)
                    .transpose(1, 0, 2).reshape(128, NK * COLS))
        l1 = np.zeros((128, body.shape[1] + pad), np_stream)
        l1[:, :body.shape[1]] = body
        l1bt = np.ascontiguousarray(L1_b[sl].reshape(COLS // 128, 128).T)
        # l2[p, 32j + n] = L2slice[128j + p, n]
        l2 = np.ascontiguousarray(
            L2_w[sl, :].reshape(COLS // 128, 128, N_OUT)
            .transpose(1, 0, 2).reshape(128, (COLS // 128) * N_OUT))
        in_maps.append(dict(at=at, xt=xt, w1=w1, b1=b1v, w2=w2, b2=b2v,
                            l1w=l1, l1bt=l1bt, l2w=l2))
    return in_maps


def kernel(**inputs):
    global LAST_RESULT
    mode = MODE
    if mode not in _program_cache:
        _program_cache[mode] = _build(mode)
    nc = _program_cache[mode]

    in_maps = _prep_inputs(
        inputs["x"], inputs["edge"], inputs["W1"], inputs["b1"],
        inputs["W2"], inputs["b2"], inputs["L1_w"], inputs["L1_b"],
        inputs["L2_w"], mode)

    res = run_bass_kernel_spmd(
        nc, in_maps, core_ids=list(range(N_CORES)), trace=TRACE)
    LAST_RESULT = res

    partial = np.zeros(N_OUT, np.float64)
    for r in res.results:
        partial += r["out"].reshape(-1).astype(np.float64)
    logits = partial.astype(np.float32) + np.asarray(inputs["L2_b"], np.float32)
    return (1.0 / (1.0 + np.exp(-logits))).astype(np.float32)



# revision 32
# speedup vs baseline: 3.9021x; 3.9021x over previous
"""Trainium2 Bass kernel for nn_BaseModel_7885559955990 (gnn_message_passing).

Model: 2 tiny GCN layers on a 1024-node graph -> flatten to v[16384] ->
relu(v @ L1_w[16384,16384] + L1_b) -> sigmoid(. @ L2_w[16384,32] + L2_b).

Distribution (8 cores, tensor-parallel per the sharding hint):
  - L1_w is sharded column-wise: core c computes v1_c = relu(v @ L1_w[:, c*2048:(c+1)*2048] + b_c)
  - L2_w is sharded row-wise:    core c computes partial_c = v1_c @ L2_w[c*2048:(c+1)*2048, :]
  - unshard = sum partials over cores, + L2_b, sigmoid  (32 floats, done host-side)
  - GCN layers are tiny and replicated on every core.

The graph operator (degree-normalized adjacency with self loops) depends only
on the edge-list input; it is densified host-side into AT[s, d] (4 MB) so the
message-passing aggregation runs as dense matmuls on the tensor engine.

The dominant cost is streaming the 128 MiB/core L1 slice from HBM
(~360 GB/s/core). The vector-matrix product uses v-chunks as the [128,1]
stationary operand so the PE streams weight columns at line rate.

Precision modes for the L1 stream (MODE):
  fp32  - exact; PE-bound (fp32 streams at 4 cyc/row): ~440 us
  f32r  - fp32 data, single-pass reduced-precision matmul: ~DMA roofline
  bf16  - bf16 weights: half the HBM traffic, ~2x faster than roofline
  split - W and v split into bf16 hi+lo pairs (3 matmul passes); same HBM
          bytes as fp32 but full-rate streaming -> DMA roofline with ~1e-6 err
"""

import numpy as np
import ml_dtypes
from contextlib import ExitStack

import concourse.bacc as bacc
import concourse.tile as tile
from concourse import mybir
from concourse.bass_utils import run_bass_kernel_spmd

F32 = mybir.dt.float32
F32R = mybir.dt.float32r
BF16 = mybir.dt.bfloat16
AF = mybir.ActivationFunctionType

N_CORES = 8
N_NODES = 1024
C = 16                    # GCN channel width
M = N_NODES * C           # 16384 flattened width
COLS = M // N_CORES       # 2048 L1 columns per core
N_OUT = 32
NK = M // 128             # 128 contraction chunks of 128

MODE = "e4dr"             # default; see module docstring
TRACE = False             # set True (module-level) to profile; result in LAST_RESULT
LAST_RESULT = None

E4 = mybir.dt.float8e4
NPAIR = (16384 // 8) // 128 // 2 * 8  # 64 k-chunk pairs (NK // 2)
SV = 256.0                # v pre-scale so fp8(v) stays in e4m3 normal range
WMAX = 120.0              # per-column |W| target after scaling (e4m3 max 240)
E4_PAD = 8192             # bytes of per-row pad to break DRAM bank aliasing
E4_CPD = 2                # k-chunk pairs per DMA (8KB/row chunks)

_MODE_CFG = {
    #        stream_dt, agg_dt, cpd (128-row chunks per DMA), split
    "fp32":  (F32,  F32,  2, False),
    "f32r":  (F32R, F32,  2, False),
    "bf16":  (BF16, BF16, 4, False),
    "split": (BF16, F32,  2, True),
}

# All DRAM tensors streamed at rate are pre-tiled on the host into
# partition-major [128, ...] layout so every dma_start is a plain 2D AP —
# 3D/rearranged APs defeat the 16-engine descriptor spray (measured
# 17 GB/s vs 287 GB/s per core).

_program_cache = {}


def _build(mode, repeat=1):
    # repeat > 1 duplicates the weight-stream phase (timing builds only):
    # wall-slope between two repeat values isolates the steady-state
    # stream+matmul rate, cancelling RPC overhead and kernel prefix/tail.
    if mode == "e4dr":
        return _build_e4dr(repeat)
    stream_dt, agg_dt, cpd, split = _MODE_CFG[mode]
    np_stream = ml_dtypes.bfloat16 if stream_dt == BF16 else np.float32
    np_agg = ml_dtypes.bfloat16 if agg_dt == BF16 else np.float32

    nc = bacc.Bacc("TRN2", target_bir_lowering=False, debug=False,
                   num_devices=N_CORES)

    # ---- DRAM tensors (per-core views; replicated unless noted).
    # at/l1w/l2w are host-pre-tiled partition-major (see _prep_inputs).
    at = nc.dram_tensor("at", [128, 8 * N_NODES], agg_dt, kind="ExternalInput").ap()
    xt = nc.dram_tensor("xt", [C, N_NODES], F32, kind="ExternalInput").ap()
    w1 = nc.dram_tensor("w1", [C, C], F32, kind="ExternalInput").ap()
    b1 = nc.dram_tensor("b1", [C, 1], F32, kind="ExternalInput").ap()
    w2 = nc.dram_tensor("w2", [C, C], F32, kind="ExternalInput").ap()
    b2 = nc.dram_tensor("b2", [C, 1], F32, kind="ExternalInput").ap()
    sub = 2 if split else 1       # sub-chunks (hi/lo) per 128-row chunk
    # +8 KB pad per partition row: a power-of-two row stride aliases DRAM
    # banks (measured 228 -> 384 GB/s/core on the 128 MB stream)
    pad = 8192 // (2 if stream_dt == BF16 else 4)
    l1w = nc.dram_tensor("l1w", [128, NK * sub * COLS + pad], stream_dt,
                         kind="ExternalInput").ap()
    l1bt = nc.dram_tensor("l1bt", [128, COLS // 128], F32, kind="ExternalInput").ap()
    l2w = nc.dram_tensor("l2w", [128, (COLS // 128) * N_OUT], F32,
                         kind="ExternalInput").ap()
    out = nc.dram_tensor("out", [1, N_OUT], F32, kind="ExternalOutput").ap()

    n_vj = COLS // 128            # 16 v1 chunks
    n_ng = COLS // 512            # 4 psum bank groups for the big matmul
    ndma = NK // cpd              # big-stream DMA count

    with tile.TileContext(nc) as tc, ExitStack() as ctx:
        const = ctx.enter_context(tc.tile_pool(name="const", bufs=1))
        small = ctx.enter_context(tc.tile_pool(name="small", bufs=1))
        wpool = ctx.enter_context(tc.tile_pool(name="wpool", bufs=8))
        dpool = ctx.enter_context(tc.tile_pool(name="dpool", bufs=1, space="DRAM"))

        # ---- constant loads (issued first so they beat the weight stream
        # into the DMA queues)
        at_sb = const.tile([128, 8 * N_NODES], agg_dt, tag="at")
        nc.sync.dma_start(at_sb[:, :], at)
        xt_sb = const.tile([C, N_NODES], F32, tag="xt")
        nc.sync.dma_start(xt_sb[:, :], xt)
        w1_sb = const.tile([C, C], F32, tag="w1")
        nc.sync.dma_start(w1_sb[:, :], w1)
        b1_sb = const.tile([C, 1], F32, tag="b1")
        nc.sync.dma_start(b1_sb[:, :], b1)
        w2_sb = const.tile([C, C], F32, tag="w2")
        nc.sync.dma_start(w2_sb[:, :], w2)
        b2_sb = const.tile([C, 1], F32, tag="b2")
        nc.sync.dma_start(b2_sb[:, :], b2)
        l1bt_sb = const.tile([128, n_vj], F32, tag="l1bt")
        nc.sync.dma_start(l1bt_sb[:, :], l1bt)
        l2w_sb = const.tile([128, n_vj * N_OUT], F32, tag="l2w")
        nc.sync.dma_start(l2w_sb[:, :], l2w)

        # ---- GCN: two layers of  hT' = relu( (AT.T-aggregated (h W)) + b )
        # h is kept transposed: [16 channels (partitions), 1024 nodes].
        def gcn_layer(h_in, w_sb, b_sb, psz, psh, zpool, hpool, li):
            # z = h @ W, built node-tile-major: z_i [128 nodes, 16]
            z_tiles = []
            for i in range(8):
                zps = psz.tile([128, C], F32, tag="zps")
                nc.tensor.matmul(zps[:, :], h_in[:, 128 * i:128 * (i + 1)],
                                 w_sb[:, :], start=True, stop=True)
                z_sb = zpool.tile([128, C], agg_dt, tag=f"z{li}_{i}")
                nc.vector.tensor_copy(z_sb[:, :], zps[:, :])
                z_tiles.append(z_sb)
            # aggregate: outT[c, d] = sum_s z[s, c] * AT[s, d]
            hps = psh.tile([C, N_NODES], F32, tag="hps")
            for i in range(8):
                for hh in range(2):
                    nc.tensor.matmul(
                        hps[:, 512 * hh:512 * (hh + 1)],
                        z_tiles[i][:, :],
                        at_sb[:, 1024 * i + 512 * hh:1024 * i + 512 * (hh + 1)],
                        start=(i == 0), stop=(i == 7),
                    )
            h_out = hpool.tile([C, N_NODES], F32, tag=f"h{li}")
            nc.scalar.activation(h_out[:, :], hps[:, :], AF.Relu, bias=b_sb[:, :])
            return h_out

        with tc.tile_pool(name="psz", bufs=2, space="PSUM") as psz, \
             tc.tile_pool(name="psh", bufs=2, space="PSUM") as psh, \
             tc.tile_pool(name="zpool", bufs=1) as zpool, \
             tc.tile_pool(name="hpool", bufs=1) as hpool:
            h1 = gcn_layer(xt_sb, w1_sb, b1_sb, psz, psh, zpool, hpool, 1)
            h2 = gcn_layer(h1, w2_sb, b2_sb, psz, psh, zpool, hpool, 2)

            # ---- vcol: v-chunks as stationary columns. vcol[16a+c, k] = v[128k+16a+c]
            # = h2[8k+a, c] = h2T[c, 8k+a]
            vcol = small.tile([128, NK], F32, tag="vcol")
            h2v = h2[:, :].rearrange("c (k a) -> c k a", a=8)
            for a in range(8):
                nc.gpsimd.dma_start(vcol[16 * a:16 * (a + 1), :], h2v[:, :, a])

        if split:
            vhi = small.tile([128, NK], BF16, tag="vhi")
            nc.vector.tensor_copy(vhi[:, :], vcol[:, :])
            vhi_f = small.tile([128, NK], F32, tag="vhif")
            nc.vector.tensor_copy(vhi_f[:, :], vhi[:, :])
            vlo_f = small.tile([128, NK], F32, tag="vlof")
            nc.vector.tensor_sub(vlo_f[:, :], vcol[:, :], vhi_f[:, :])
            vlo = small.tile([128, NK], BF16, tag="vlo")
            nc.vector.tensor_copy(vlo[:, :], vlo_f[:, :])
            # passes: (stationary vec, hi/lo weight sub-chunk)
            passes = [(vhi, 0), (vlo, 0), (vhi, 1)]
        elif stream_dt == F32:
            passes = [(vcol, 0)]
        else:
            vs = small.tile([128, NK], stream_dt, tag="vs")
            nc.vector.tensor_copy(vs[:, :], vcol[:, :])
            passes = [(vs, 0)]

        # ---- big matmul: vps[0, n] = sum_k v[k] * L1[k, n]
        with tc.tile_pool(name="psv", bufs=1, space="PSUM") as psv, \
             tc.tile_pool(name="ps32", bufs=1, space="PSUM") as ps32:
            vps = psv.tile([1, COLS], F32, tag="vps")
            wfree = COLS * sub * cpd     # tile free elems per DMA
            for rep in range(repeat):
                for t in range(ndma):
                    wt = wpool.tile([128, wfree], stream_dt, tag="w")
                    nc.sync.dma_start(wt[:, :], l1w[:, wfree * t:wfree * (t + 1)])
                    for cc in range(cpd):
                        k = cpd * t + cc
                        for j in range(n_ng):
                            for si, (vv, wi) in enumerate(passes):
                                base = (sub * cc + wi) * 2048
                                nc.tensor.matmul(
                                    vps[0:1, 512 * j:512 * (j + 1)],
                                    vv[:, k:k + 1],
                                    wt[:, base + 512 * j:base + 512 * (j + 1)],
                                    start=(k == 0 and si == 0 and rep == 0),
                                    stop=(k == NK - 1 and si == len(passes) - 1
                                          and rep == repeat - 1),
                                )

            # ---- tail: v1 = relu(vps + b), re-laid out to [128, 16] via DRAM bounce
            v1row = small.tile([1, COLS], F32, tag="v1row")
            nc.scalar.copy(v1row[:, :], vps[0:1, :])
            dscratch = dpool.tile([1, COLS], F32, tag="dscratch")
            nc.gpsimd.dma_start(dscratch[:, :], v1row[:, :])
            v1t = small.tile([128, n_vj], F32, tag="v1t")
            nc.gpsimd.dma_start(
                v1t[:, :], dscratch[:, :].rearrange("o (j p) -> p (o j)", p=128))
            nc.vector.tensor_add(v1t[:, :], v1t[:, :], l1bt_sb[:, :])
            nc.vector.tensor_relu(v1t[:, :], v1t[:, :])

            # ---- second matmul: partial[1, 32] = sum_j v1_j^T @ L2_j
            p32 = ps32.tile([1, N_OUT], F32, tag="p32")
            for j in range(n_vj):
                nc.tensor.matmul(
                    p32[0:1, :], v1t[:, j:j + 1],
                    l2w_sb[:, N_OUT * j:N_OUT * (j + 1)],
                    start=(j == 0), stop=(j == n_vj - 1),
                )
            out_sb = small.tile([1, N_OUT], F32, tag="out")
            nc.vector.tensor_copy(out_sb[:, :], p32[0:1, :])
            nc.sync.dma_start(out, out_sb[:, :])

    nc.compile()
    return nc


def _build_e4dr(repeat=1, debug=False):
    """fp8-e4m3 DoubleRow single-pass stream.

    W is host-packed per 256-row pair-group g: l1w[p, 4096g + 2n + t] =
    Wq[256g + 128t + p, n], so each DoubleRow matmul contracts 256 rows
    (two 128-chunks) with the stationary v pair vq[:, 2g:2g+2].
    Halves both HBM bytes (1B/elt) and PE row count (0.5 cyc/row) vs bf16.

    Tail avoids the DRAM bounce: 16 K=1 matmuls against a ones[1,1] rhs
    transpose the [1,2048] psum row into [128,16] partition-major form.
    """
    nc = bacc.Bacc("TRN2", target_bir_lowering=False, debug=False,
                   num_devices=N_CORES)

    # adjacency pre-scaled x16 into e4m3 normal range; the 1/16 is folded
    # into each layer's activation scale
    at = nc.dram_tensor("at", [128, 8 * N_NODES], E4, kind="ExternalInput").ap()
    xt = nc.dram_tensor("xt", [C, N_NODES], F32, kind="ExternalInput").ap()
    w1 = nc.dram_tensor("w1", [C, C], F32, kind="ExternalInput").ap()
    b1 = nc.dram_tensor("b1", [C, 1], F32, kind="ExternalInput").ap()
    w2 = nc.dram_tensor("w2", [C, C], F32, kind="ExternalInput").ap()
    b2s = nc.dram_tensor("b2s", [C, 1], F32, kind="ExternalInput").ap()
    l1w = nc.dram_tensor("l1w", [128, NPAIR * 2 * COLS + E4_PAD], E4,
                         kind="ExternalInput").ap()
    l1bt = nc.dram_tensor("l1bt", [128, COLS // 128], F32, kind="ExternalInput").ap()
    l2w = nc.dram_tensor("l2w", [128, (COLS // 128) * N_OUT], F32,
                         kind="ExternalInput").ap()
    one = nc.dram_tensor("one", [1, 1], F32, kind="ExternalInput").ap()
    ident = nc.dram_tensor("ident", [C, C], F32, kind="ExternalInput").ap()
    out = nc.dram_tensor("out", [1, N_OUT], F32, kind="ExternalOutput").ap()
    if debug:
        dbg_vcol = nc.dram_tensor("dbg_vcol", [128, NK], F32,
                                  kind="ExternalOutput").ap()
        dbg_v1t = nc.dram_tensor("dbg_v1t", [128, COLS // 128], F32,
                                 kind="ExternalOutput").ap()
        dbg_h2 = nc.dram_tensor("dbg_h2", [C, N_NODES], F32,
                                kind="ExternalOutput").ap()

    n_vj = COLS // 128            # 16 v1 chunks / transpose matmuls
    n_ng = COLS // 512            # 4 psum bank groups for the big matmul
    ndma = NPAIR // E4_CPD        # 32 stream DMAs
    wfree = E4_CPD * 2 * COLS     # free elems per chunk (8192)

    with tile.TileContext(nc) as tc, ExitStack() as ctx:
        const = ctx.enter_context(tc.tile_pool(name="const", bufs=1))
        small = ctx.enter_context(tc.tile_pool(name="small", bufs=1))
        wpool = ctx.enter_context(tc.tile_pool(name="wpool", bufs=14))

        at_sb = const.tile([128, 8 * N_NODES], E4, tag="at")
        nc.sync.dma_start(at_sb[:, :], at)
        xt_sb = const.tile([C, N_NODES], F32, tag="xt")
        nc.sync.dma_start(xt_sb[:, :], xt)
        w1_sb = const.tile([C, C], F32, tag="w1")
        nc.sync.dma_start(w1_sb[:, :], w1)
        b1_sb = const.tile([C, 1], F32, tag="b1")
        nc.sync.dma_start(b1_sb[:, :], b1)
        w2_sb = const.tile([C, C], F32, tag="w2")
        nc.sync.dma_start(w2_sb[:, :], w2)
        b2_sb = const.tile([C, 1], F32, tag="b2s")
        nc.sync.dma_start(b2_sb[:, :], b2s)
        l1bt_sb = const.tile([128, n_vj], F32, tag="l1bt")
        nc.sync.dma_start(l1bt_sb[:, :], l1bt)
        l2w_sb = const.tile([128, n_vj * N_OUT], F32, tag="l2w")
        nc.sync.dma_start(l2w_sb[:, :], l2w)
        one_sb = const.tile([1, 1], F32, tag="one")
        nc.sync.dma_start(one_sb[:, :], one)
        ident_sb = const.tile([C, C], F32, tag="ident")
        nc.sync.dma_start(ident_sb[:, :], ident)

        def gcn_layer(h_in, w_sb, b_sb, psz, psh, zpool, hpool, li, scale):
            z_tiles = []
            for i in range(8):
                zps = psz.tile([128, C], F32, tag="zps")
                nc.tensor.matmul(zps[:, :], h_in[:, 128 * i:128 * (i + 1)],
                                 w_sb[:, :], start=True, stop=True)
                z_sb = zpool.tile([128, C], E4, tag=f"z{li}_{i}")
                nc.vector.tensor_copy(z_sb[:, :], zps[:, :])
                z_tiles.append(z_sb)
            hps = psh.tile([C, N_NODES], F32, tag="hps")
            for i in range(8):
                for hh in range(2):
                    nc.tensor.matmul(
                        hps[:, 512 * hh:512 * (hh + 1)],
                        z_tiles[i][:, :],
                        at_sb[:, 1024 * i + 512 * hh:1024 * i + 512 * (hh + 1)],
                        start=(i == 0), stop=(i == 7),
                    )
            h_out = hpool.tile([C, N_NODES], F32, tag=f"h{li}")
            nc.scalar.activation(h_out[:, :], hps[:, :], AF.Relu,
                                 bias=b_sb[:, :], scale=scale)
            return h_out

        with tc.tile_pool(name="psz", bufs=2, space="PSUM") as psz, \
             tc.tile_pool(name="psh", bufs=2, space="PSUM") as psh, \
             tc.tile_pool(name="zpool", bufs=1) as zpool, \
             tc.tile_pool(name="hpool", bufs=1) as hpool:
            # activation scale folds the 1/16 adjacency pre-scale back out;
            # layer 2 additionally applies SV so fp8(v) is in e4m3 range
            h1 = gcn_layer(xt_sb, w1_sb, b1_sb, psz, psh, zpool, hpool, 1,
                           1.0 / 16.0)
            h2 = gcn_layer(h1, w2_sb, b2_sb, psz, psh, zpool, hpool, 2,
                           SV / 16.0)

            # vcol[8c+r, k] = h2[c, 128r+k] = v[2048r + 16k + c]: contiguous
            # 512B runs (128 descriptors). The k-chunk order is a permutation
            # of the natural one; the host W pack uses the same row order.
            # vcol[q, 16r+c] = h2[c, 128r+q] via 8 PE transposes -> one psum
            # tile; chunk kappa = 16r+c <-> W row 2048r + 16q + c (host pack)
            vcol = small.tile([128, NK], F32, tag="vcol")
            with tc.tile_pool(name="psvt", bufs=1, space="PSUM") as psvt:
                tvps = psvt.tile([128, NK], F32, tag="tvps")
                for r in range(8):
                    nc.tensor.transpose(tvps[:, 16 * r:16 * (r + 1)],
                                        h2[:, 128 * r:128 * (r + 1)],
                                        ident_sb[:, :])
                nc.vector.tensor_copy(vcol[:, :], tvps[:, :])
            if debug:
                nc.sync.dma_start(dbg_h2, h2[:, :])
                nc.sync.dma_start(dbg_vcol, vcol[:, :])

        vq = small.tile([128, NK], E4, tag="vq")
        nc.vector.tensor_copy(vq[:, :], vcol[:, :])
        # pair view: pair g = (chunk g, chunk 64+g) -> ldweights AP
        # [p][t:2][g:1] with 64B pair step at dim1 (BIR DoubleRow rule)
        vqp = vq[:, :].rearrange("p (t g) -> p t g", t=2)

        with tc.tile_pool(name="psv", bufs=1, space="PSUM") as psv, \
             tc.tile_pool(name="pst", bufs=1, space="PSUM") as pst, \
             tc.tile_pool(name="ps32", bufs=1, space="PSUM") as ps32:
            vps = psv.tile([1, COLS], F32, tag="vps")
            for rep in range(repeat):
                for t in range(ndma):
                    wt = wpool.tile([128, wfree], E4, tag="w")
                    nc.sync.dma_start(wt[:, :], l1w[:, wfree * t:wfree * (t + 1)])
                    for cc in range(E4_CPD):
                        g = E4_CPD * t + cc
                        # DoubleRow ifmap AP: [p][pair: num 2, step 2048][n:512]
                        pair = (wt[:, 4096 * cc:4096 * (cc + 1)]
                                .rearrange("p (t n) -> p t n", t=2))
                        for j in range(n_ng):
                            nc.tensor.matmul(
                                vps[0:1, 512 * j:512 * (j + 1)],
                                vqp[:, :, g:g + 1],
                                pair[:, :, 512 * j:512 * (j + 1)],
                                start=(g == 0 and rep == 0),
                                stop=(g == NPAIR - 1 and rep == repeat - 1),
                                perf_mode=mybir.MatmulPerfMode.DoubleRow,
                            )

            # ---- tail: evacuate psum row (two engines), transpose via
            # 16 K=1 matmuls, bias+relu, L2 matmul
            vsb = small.tile([1, COLS], F32, tag="vsb")
            nc.scalar.copy(vsb[:, 0:COLS // 2], vps[0:1, 0:COLS // 2])
            nc.vector.tensor_copy(vsb[:, COLS // 2:], vps[0:1, COLS // 2:])
            tps = pst.tile([128, n_vj], F32, tag="tps")
            for j in range(n_vj):
                nc.tensor.matmul(tps[:, j:j + 1],
                                 vsb[0:1, 128 * j:128 * (j + 1)],
                                 one_sb[0:1, 0:1], start=True, stop=True)
            v1t = small.tile([128, n_vj], F32, tag="v1t")
            nc.vector.tensor_copy(v1t[:, :], tps[:, :])
            nc.vector.tensor_add(v1t[:, :], v1t[:, :], l1bt_sb[:, :])
            nc.vector.tensor_relu(v1t[:, :], v1t[:, :])
            if debug:
                nc.sync.dma_start(dbg_v1t, v1t[:, :])

            p32 = ps32.tile([1, N_OUT], F32, tag="p32")
            for j in range(n_vj):
                nc.tensor.matmul(
                    p32[0:1, :], v1t[:, j:j + 1],
                    l2w_sb[:, N_OUT * j:N_OUT * (j + 1)],
                    start=(j == 0), stop=(j == n_vj - 1),
                )
            out_sb = small.tile([1, N_OUT], F32, tag="out")
            nc.vector.tensor_copy(out_sb[:, :], p32[0:1, :])
            nc.sync.dma_start(out, out_sb[:, :])

    nc.compile()
    return nc


def _prep_e4dr(x, edge, W1, b1, W2, b2, L1_w, L1_b, L2_w):
    e4 = ml_dtypes.float8_e4m3
    at = (_host_adjacency(edge) * np.float32(16.0)).astype(e4)
    at = np.ascontiguousarray(
        at.reshape(8, 128, N_NODES).transpose(1, 0, 2).reshape(128, 8 * N_NODES))
    xt = np.ascontiguousarray(np.asarray(x, np.float32).T)
    w1 = np.ascontiguousarray(np.asarray(W1, np.float32))
    b1v = np.asarray(b1, np.float32).reshape(C, 1).copy()
    w2 = np.ascontiguousarray(np.asarray(W2, np.float32))
    b2v = (np.asarray(b2, np.float32) * SV).reshape(C, 1).copy()
    L1_w = np.asarray(L1_w, np.float32)
    L1_b = np.asarray(L1_b, np.float32)
    L2_w = np.asarray(L2_w, np.float32)
    onev = np.ones((1, 1), np.float32)
    identv = np.eye(C, dtype=np.float32)

    in_maps = []
    for cix in range(N_CORES):
        sl = slice(COLS * cix, COLS * (cix + 1))
        wsl = L1_w[:, sl]
        cmax = np.abs(wsl).max(axis=0)
        csc = (WMAX / np.maximum(cmax, 1e-30)).astype(np.float32)
        q = (wsl * csc[None, :]).astype(e4)
        # k-chunk row order matches the on-device vcol gather:
        # chunk k, partition p  <->  W row 2048*(p%8) + 16*k + p//8.
        # l1[p, 4096g + 2048t + n] = q[row(p, 2g+t), n]  (plane-major pairs)
        pix = np.arange(128)
        kix = np.arange(NK)
        # chunk kappa, partition q <-> W row 2048*(kappa//16) + 16*q + kappa%16
        rowidx = (2048 * (kix[None, :] // 16) + 16 * pix[:, None]
                  + kix[None, :] % 16)
        # pair g pairs vq columns (g, 64+g): body[p, 4096g+2048t+n] =
        # qperm[p, g + 64t, n]
        body = (q[rowidx].reshape(128, 2, NPAIR, COLS).transpose(0, 2, 1, 3)
                .reshape(128, NPAIR * 2 * COLS))
        l1 = np.zeros((128, NPAIR * 2 * COLS + E4_PAD), e4)
        l1[:, :body.shape[1]] = body
        bsc = (L1_b[sl] * SV * csc).astype(np.float32)
        l1bt = np.ascontiguousarray(bsc.reshape(COLS // 128, 128).T)
        l2s = (L2_w[sl, :] / (SV * csc)[:, None]).astype(np.float32)
        l2 = np.ascontiguousarray(
            l2s.reshape(COLS // 128, 128, N_OUT)
            .transpose(1, 0, 2).reshape(128, (COLS // 128) * N_OUT))
        in_maps.append(dict(at=at, xt=xt, w1=w1, b1=b1v, w2=w2, b2s=b2v,
                            l1w=l1, l1bt=l1bt, l2w=l2, one=onev, ident=identv))
    return in_maps


def _host_adjacency(edge):
    """Dense AT[s, d] = sum over (self-looped, deg-normalized) edges s->d."""
    src = edge[0].astype(np.int64)
    dst = edge[1].astype(np.int64)
    loop = np.arange(N_NODES, dtype=np.int64)
    s = np.concatenate([src, loop])
    d = np.concatenate([dst, loop])
    deg = np.bincount(d, minlength=N_NODES).astype(np.float32)
    dinv = np.where(deg > 0, deg, np.float32(1.0)) ** np.float32(-0.5)
    norm = (dinv[s] * dinv[d]).astype(np.float32)
    at = np.zeros((N_NODES, N_NODES), np.float32)
    np.add.at(at, (s, d), norm)
    return at


def _prep_inputs(x, edge, W1, b1, W2, b2, L1_w, L1_b, L2_w, mode):
    stream_dt, agg_dt, cpd, split = _MODE_CFG[mode]
    np_stream = ml_dtypes.bfloat16 if stream_dt == BF16 else np.float32
    np_agg = ml_dtypes.bfloat16 if agg_dt == BF16 else np.float32

    # partition-major tiling: AT [1024,1024] -> [128, 8*1024] with
    # at_t[p, 1024*i + d] = AT[128*i + p, d]
    at = _host_adjacency(edge).astype(np_agg)
    at = np.ascontiguousarray(
        at.reshape(8, 128, N_NODES).transpose(1, 0, 2).reshape(128, 8 * N_NODES))
    xt = np.ascontiguousarray(np.asarray(x, np.float32).T)
    w1 = np.ascontiguousarray(np.asarray(W1, np.float32))
    b1v = np.asarray(b1, np.float32).reshape(C, 1).copy()
    w2 = np.ascontiguousarray(np.asarray(W2, np.float32))
    b2v = np.asarray(b2, np.float32).reshape(C, 1).copy()
    L1_w = np.asarray(L1_w, np.float32)
    L1_b = np.asarray(L1_b, np.float32)
    L2_w = np.asarray(L2_w, np.float32)

    in_maps = []
    for c in range(N_CORES):
        sl = slice(COLS * c, COLS * (c + 1))
        wsl = np.ascontiguousarray(L1_w[:, sl])
        pad = 8192 // (2 if np_stream == ml_dtypes.bfloat16 else 4)
        if split:
            hi = wsl.astype(ml_dtypes.bfloat16)
            lo = (wsl - hi.astype(np.float32)).astype(ml_dtypes.bfloat16)
            # partition-major, k-major then hi/lo:
            # l1[p, (2k+s)*2048 + n] = (hi if s==0 else lo)[128k+p, n]
            body = np.empty((NK, 2, 128, COLS), ml_dtypes.bfloat16)
            body[:, 0] = hi.reshape(NK, 128, COLS)
            body[:, 1] = lo.reshape(NK, 128, COLS)
            body = body.transpose(2, 0, 1, 3).reshape(128, NK * 2 * COLS)
        else:
            # l1[p, 2048k + n] = Wslice[128k + p, n]
            body = (wsl.astype(np_stream).reshape(NK, 128, COLS)
                    .transpose(1, 0, 2).reshape(128, NK * COLS))
        l1 = np.zeros((128, body.shape[1] + pad), np_stream)
        l1[:, :body.shape[1]] = body
        l1bt = np.ascontiguousarray(L1_b[sl].reshape(COLS // 128, 128).T)
        # l2[p, 32j + n] = L2slice[128j + p, n]
        l2 = np.ascontiguousarray(
            L2_w[sl, :].reshape(COLS // 128, 128, N_OUT)
            .transpose(1, 0, 2).reshape(128, (COLS // 128) * N_OUT))
        in_maps.append(dict(at=at, xt=xt, w1=w1, b1=b1v, w2=w2, b2=b2v,
                            l1w=l1, l1bt=l1bt, l2w=l2))
    return in_maps


def kernel(**inputs):
    global LAST_RESULT
    mode = MODE
    if mode not in _program_cache:
        _program_cache[mode] = _build(mode)
    nc = _program_cache[mode]

    if mode == "e4dr":
        in_maps = _prep_e4dr(
            inputs["x"], inputs["edge"], inputs["W1"], inputs["b1"],
            inputs["W2"], inputs["b2"], inputs["L1_w"], inputs["L1_b"],
            inputs["L2_w"])
    else:
        in_maps = _prep_inputs(
            inputs["x"], inputs["edge"], inputs["W1"], inputs["b1"],
            inputs["W2"], inputs["b2"], inputs["L1_w"], inputs["L1_b"],
            inputs["L2_w"], mode)

    res = run_bass_kernel_spmd(
        nc, in_maps, core_ids=list(range(N_CORES)), trace=TRACE)
    LAST_RESULT = res

    partial = np.zeros(N_OUT, np.float64)
    for r in res.results:
        partial += r["out"].reshape(-1).astype(np.float64)
    logits = partial.astype(np.float32) + np.asarray(inputs["L2_b"], np.float32)
    return (1.0 / (1.0 + np.exp(-logits))).astype(np.float32)

